# revision 5
# baseline (speedup 1.0000x reference)
"""Differentiable palette quantization on 8 Trainium2 NeuronCores.

Math: for each image b, pixel x, palette p_k (k=64):
    w = softmax_k(-|x - p_k|^2 / T);  out = sum_k w_k p_k
Softmax is invariant to the per-pixel |x|^2 term, so the logit reduces to
    (2*dot(x, p_k) - |p_k|^2) / T = (2/T) * dot([x, 1], [p_k, -|p_k|^2/2])
which is a K=4 matmul.  Numerator (sum_k e_k p_k) and denominator
(sum_k e_k) come from a second matmul against [palette | ones].

Sharding: pure data parallel, 2 images per core.  The two images of a core
are stacked on partitions (64+64 palette entries) and share the pixel
stream via a block-diagonal K=8 stationary matrix.  Pixels are split into
4 quarters mapped to the four 32-row strips of the PE array
(tile_position row tiling) so four N=512 matmuls run concurrently and the
input DMA spreads over 4 SBUF port groups.

Pipeline per round (512 pixel-pairs per quarter = 2048 psum columns):
  PE  : 4x block-diag matmul (fp32r)          -> psum1[128, 2048]
  ACT : exp(scale * psum1) (2 ops, FD=1024)   -> e[128, 2048]
  PE  : 16x (e-block[128,128])^T @ palW[128,8] -> psum2[128, 8] slices
Every 4 rounds psum2[128, 512] is full; DVE divides numerators by
denominators and the [128, 192]-per-image result is DMA'd out densely.
Host reorders the output blocks back to image layout (pure reshape).
"""

import os
import sys

for _p in ("/opt/trn_rl_repo", os.path.expanduser("~/.axon_site/_ro/trn_rl_repo")):
    if os.path.isdir(_p) and _p not in sys.path:
        sys.path.insert(0, _p)

import numpy as np

import concourse.bass as bass
import concourse.tile as tile
from concourse import bacc, mybir
from concourse.bass_utils import run_bass_kernel_spmd

# problem constants (hardcoded per contract)
B, H, W, C, K = 16, 256, 256, 3, 64
NCORES = 8
IMGS_PER_CORE = B // NCORES            # 2
P = H * W                              # 65536 pixel-pairs per core
NQ = 4                                 # PE row-tile quarters
QP = P // NQ                           # 16384 pixels per quarter
RN = 512                               # pixels per quarter per round
ROUNDS = QP // RN                      # 32
RPG = 4                                # rounds per output supergroup
NG = ROUNDS // RPG                     # 8 supergroups

# tuning knobs (env-overridable for experiments)
MM1_DT = os.environ.get("PALQ_MM1_DT", "float32r")   # float32 | float32r
E_DT = os.environ.get("PALQ_E_DT", "bfloat16")       # float32 | bfloat16
PALW_SPLIT = os.environ.get("PALQ_PALW_SPLIT", "1") == "1"  # hi/lo palW fix-up


def _dt(name):
    return getattr(mybir.dt, name)


def build_bass(scale: float):
    nc = bacc.Bacc("TRN2", target_bir_lowering=False, debug=False)
    f32 = mybir.dt.float32
    e_dt = _dt(E_DT)
    mm1_dt = _dt(MM1_DT)

    # fp32r operands must be produced as fp32r end-to-end (BIR verifier)
    xin = nc.dram_tensor("xin", [NQ, 8, QP], mm1_dt, kind="ExternalInput")
    palt = nc.dram_tensor("palt", [8, 128], mm1_dt, kind="ExternalInput")
    palw_hi = nc.dram_tensor("palw_hi", [128, 8], e_dt, kind="ExternalInput")
    n_palw = 2 if (PALW_SPLIT and E_DT == "bfloat16") else 1
    palw_lo = (
        nc.dram_tensor("palw_lo", [128, 8], e_dt, kind="ExternalInput")
        if n_palw == 2
        else None
    )
    out = nc.dram_tensor("out", [IMGS_PER_CORE, NG, 128, 192], f32,
                         kind="ExternalOutput")

    with tile.TileContext(nc) as tc:
        import contextlib
        with contextlib.ExitStack() as ctx:
            singles = ctx.enter_context(tc.tile_pool(name="singles", bufs=1))
            epool = ctx.enter_context(tc.tile_pool(name="epool", bufs=3))
            ps1 = ctx.enter_context(tc.tile_pool(name="ps1", bufs=1, space="PSUM"))
            ps2 = ctx.enter_context(tc.tile_pool(name="ps2", bufs=2, space="PSUM"))
            vpool = ctx.enter_context(tc.tile_pool(name="vpool", bufs=2))
            opool = ctx.enter_context(tc.tile_pool(name="opool", bufs=3))

            # resident input pixels: quarter j on partitions [32j, 32j+8)
            xsb = singles.tile([128, QP], mm1_dt)
            for j in range(NQ):
                for h in range(4):
                    sl = slice(h * (QP // 4), (h + 1) * (QP // 4))
                    nc.sync.dma_start(out=xsb[32 * j:32 * j + 8, sl],
                                      in_=xin.ap()[j, :, sl])

            # stationary block-diag palette, replicated into each row strip
            palt_sb = singles.tile([128, 128], mm1_dt)
            for j in range(NQ):
                nc.sync.dma_start(out=palt_sb[32 * j:32 * j + 8, :],
                                  in_=palt.ap())
            palw_sb = singles.tile([128, 8], e_dt)
            nc.sync.dma_start(out=palw_sb, in_=palw_hi.ap())
            if n_palw == 2:
                palw_lo_sb = singles.tile([128, 8], e_dt)
                nc.sync.dma_start(out=palw_lo_sb, in_=palw_lo.ap())

            for g in range(NG):
                psum2 = ps2.tile([128, 512], f32)
                for rsub in range(RPG):
                    r = g * RPG + rsub
                    psum1 = ps1.tile([128, 4 * RN], f32)
                    for j in range(NQ):
                        nc.tensor.matmul(
                            out=psum1[:, RN * j:RN * (j + 1)],
                            lhsT=palt_sb[32 * j:32 * j + 8, :],
                            rhs=xsb[32 * j:32 * j + 8, RN * r:RN * (r + 1)],
                            start=True, stop=True,
                            tile_position=(32 * j, 0),
                        )
                    e_sb = epool.tile([128, 4 * RN], e_dt)
                    for h in range(2):
                        sl = slice(h * 2 * RN, (h + 1) * 2 * RN)
                        nc.scalar.activation(
                            out=e_sb[:, sl], in_=psum1[:, sl],
                            func=mybir.ActivationFunctionType.Exp,
                            scale=float(scale),
                        )
                    for t in range(16):
                        s = 16 * rsub + t
                        lhsT = e_sb[:, 128 * t:128 * (t + 1)]
                        nc.tensor.matmul(
                            out=psum2[:, 8 * s:8 * s + 8],
                            lhsT=lhsT, rhs=palw_sb,
                            start=True, stop=(n_palw == 1),
                        )
                        if n_palw == 2:
                            nc.tensor.matmul(
                                out=psum2[:, 8 * s:8 * s + 8],
                                lhsT=lhsT, rhs=palw_lo_sb,
                                start=False, stop=True,
                            )

                # divide numerators by denominators; write per-image blocks
                psr = psum2.rearrange("p (s e) -> p s e", e=8)
                recA = vpool.tile([128, 64], f32)
                nc.vector.reciprocal(out=recA, in_=psr[:, :, 3])
                recB = vpool.tile([128, 64], f32)
                nc.vector.reciprocal(out=recB, in_=psr[:, :, 7])
                outA = opool.tile([128, 192], f32)
                outB = opool.tile([128, 192], f32)
                oA = outA.rearrange("p (u c) -> p u c", c=3)
                oB = outB.rearrange("p (u c) -> p u c", c=3)
                for c in range(3):
                    nc.vector.tensor_mul(out=oA[:, :, c], in0=psr[:, :, c],
                                         in1=recA)
                    nc.vector.tensor_mul(out=oB[:, :, c], in0=psr[:, :, 4 + c],
                                         in1=recB)
                nc.sync.dma_start(out=out.ap()[0, g], in_=outA)
                nc.sync.dma_start(out=out.ap()[1, g], in_=outB)

    nc.compile()
    return nc


def _host_prep(images, palettes):
    """Per-core input arrays. images [16,256,256,3] f32, palettes [16,64,3]."""
    import ml_dtypes

    imgs = np.ascontiguousarray(images, np.float32).reshape(B, P, C)
    pals = np.ascontiguousarray(palettes, np.float32)
    in_maps = []
    for core in range(NCORES):
        ia, ib = imgs[2 * core], imgs[2 * core + 1]
        xin = np.empty((NQ, 8, QP), np.float32)
        xq_a = ia.reshape(NQ, QP, C).transpose(0, 2, 1)
        xq_b = ib.reshape(NQ, QP, C).transpose(0, 2, 1)
        xin[:, 0:3] = xq_a
        xin[:, 3] = 1.0
        xin[:, 4:7] = xq_b
        xin[:, 7] = 1.0

        pa, pb = pals[2 * core], pals[2 * core + 1]
        palt = np.zeros((8, 128), np.float32)
        palt[0:3, 0:64] = pa.T
        palt[3, 0:64] = -0.5 * (pa * pa).sum(-1)
        palt[4:7, 64:128] = pb.T
        palt[7, 64:128] = -0.5 * (pb * pb).sum(-1)

        palw = np.zeros((128, 8), np.float32)
        palw[0:64, 0:3] = pa
        palw[0:64, 3] = 1.0
        palw[64:128, 4:7] = pb
        palw[64:128, 7] = 1.0

        m = {"xin": xin, "palt": palt}
        if E_DT == "bfloat16":
            hi = palw.astype(ml_dtypes.bfloat16)
            m["palw_hi"] = hi
            if PALW_SPLIT:
                m["palw_lo"] = (palw - hi.astype(np.float32)).astype(
                    ml_dtypes.bfloat16)
        else:
            m["palw_hi"] = palw
        in_maps.append(m)
    return in_maps


def _host_post(results):
    """results[core]["out"] [2, 8, 128, 192] -> [16, 256, 256, 3]."""
    out = np.empty((B, P, C), np.float32)
    for core in range(NCORES):
        o = results[core]["out"]
        dec = (o.reshape(IMGS_PER_CORE, NG, 128, RPG, NQ, 4, C)
                .transpose(0, 4, 1, 3, 5, 2, 6)
                .reshape(IMGS_PER_CORE, P, C))
        out[2 * core] = dec[0]
        out[2 * core + 1] = dec[1]
    return out.reshape(B, H, W, C)


_CACHE = {}


def _get_nc(scale: float):
    key = (round(float(scale), 12), MM1_DT, E_DT, PALW_SPLIT)
    if key not in _CACHE:
        _CACHE[key] = build_bass(scale)
    return _CACHE[key]


def kernel(images, palettes, temperature, _trace=False):
    scale = 2.0 / float(np.asarray(temperature))
    nc = _get_nc(scale)
    in_maps = _host_prep(images, palettes)
    res = run_bass_kernel_spmd(nc, in_maps, core_ids=list(range(NCORES)),
                               trace=_trace)
    out = _host_post(res.results)
    if _trace:
        kernel.last_result = res
    return out


# revision 16
# speedup vs baseline: 1.6432x; 1.6432x over previous
"""Differentiable palette quantization on 8 Trainium2 NeuronCores.

Math: for each image b, pixel x, palette p_k (k=64):
    w = softmax_k(-|x - p_k|^2 / T);  out = sum_k w_k p_k
Softmax is invariant to the per-pixel |x|^2 term, so the logit reduces to
    (2*dot(x, p_k) - |p_k|^2) / T = (2/T) * dot([x, 1], [p_k, -|p_k|^2/2])
which is a K=4 matmul.  Numerator (sum_k e_k p_k) and denominator
(sum_k e_k) come from a second matmul against [palette | ones].

Sharding: pure data parallel, 2 images per core.  The two images of a core
are stacked on partitions (64+64 palette entries) and share the pixel
stream via a block-diagonal K=8 stationary matrix.  Pixels are split into
4 quarters mapped to the four 32-row strips of the PE array
(tile_position row tiling) so four N=512 matmuls run concurrently and the
input DMA spreads over 4 SBUF port groups.

Pipeline per round (512 pixel-pairs per quarter = 2048 psum columns):
  PE  : 4x block-diag matmul (fp32r)          -> psum1[128, 2048]
  ACT : exp(scale * psum1) (2 ops, FD=1024)   -> e[128, 2048]
  PE  : 16x (e-block[128,128])^T @ palW[128,8] -> psum2[128, 8] slices
Every 4 rounds psum2[128, 512] is full; DVE divides numerators by
denominators and the [128, 192]-per-image result is DMA'd out densely.
Host reorders the output blocks back to image layout (pure reshape).
"""

import os
import sys

for _p in ("/opt/trn_rl_repo", os.path.expanduser("~/.axon_site/_ro/trn_rl_repo")):
    if os.path.isdir(_p) and _p not in sys.path:
        sys.path.insert(0, _p)

import numpy as np

import concourse.bass as bass
import concourse.tile as tile
from concourse import bacc, mybir
from concourse.bass_utils import run_bass_kernel_spmd

# problem constants (hardcoded per contract)
B, H, W, C, K = 16, 256, 256, 3, 64
NCORES = 8
IMGS_PER_CORE = B // NCORES            # 2
P = H * W                              # 65536 pixel-pairs per core
NQ = 4                                 # PE row-tile quarters
QP = P // NQ                           # 16384 pixels per quarter
RN = 512                               # pixels per quarter per round
ROUNDS = QP // RN                      # 32
RPG = 4                                # rounds per output supergroup
NG = ROUNDS // RPG                     # 8 supergroups

# tuning knobs (env-overridable for experiments)
MM1_DT = os.environ.get("PALQ_MM1_DT", "float16")    # float16|float32|float32r
E_DT = os.environ.get("PALQ_E_DT", "float16")        # float16|float32|bfloat16
PALW_SPLIT = os.environ.get("PALQ_PALW_SPLIT", "0") == "1"  # hi/lo palW fix-up
MM1_SPLIT = os.environ.get("PALQ_MM1_SPLIT", "0") == "1"    # hi/lo x & palT


def _dt(name):
    return getattr(mybir.dt, name)


def build_bass(scale: float):
    nc = bacc.Bacc("TRN2", target_bir_lowering=False, debug=False)
    f32 = mybir.dt.float32
    e_dt = _dt(E_DT)
    mm1_dt = _dt(MM1_DT)

    nsp = 2 if MM1_SPLIT else 1
    # fp32r operands must be produced as fp32r end-to-end (BIR verifier)
    xin = nc.dram_tensor("xin", [nsp, NQ, 8, QP], mm1_dt, kind="ExternalInput")
    palt = nc.dram_tensor("palt", [nsp, 8, 128], mm1_dt, kind="ExternalInput")
    palw_hi = nc.dram_tensor("palw_hi", [128, 8], e_dt, kind="ExternalInput")
    n_palw = 2 if (PALW_SPLIT and E_DT != "float32") else 1
    palw_lo = (
        nc.dram_tensor("palw_lo", [128, 8], e_dt, kind="ExternalInput")
        if n_palw == 2
        else None
    )
    out = nc.dram_tensor("out", [IMGS_PER_CORE, NG, 128, 192], f32,
                         kind="ExternalOutput")

    with tile.TileContext(nc) as tc:
        import contextlib
        with contextlib.ExitStack() as ctx:
            singles = ctx.enter_context(tc.tile_pool(name="singles", bufs=1))
            epool = ctx.enter_context(tc.tile_pool(name="epool", bufs=4))
            ps1 = ctx.enter_context(tc.tile_pool(name="ps1", bufs=2, space="PSUM"))
            ps2 = ctx.enter_context(tc.tile_pool(name="ps2", bufs=2, space="PSUM"))
            vpool = ctx.enter_context(tc.tile_pool(name="vpool", bufs=2))
            opool = ctx.enter_context(tc.tile_pool(name="opool", bufs=3))

            # resident input pixels: quarter j on partitions [32j, 32j+8)
            xsbs, palt_sbs = [], []
            for sp in range(nsp):
                xsb = singles.tile([128, QP], mm1_dt, name=f"xsb{sp}")
                xsbs.append(xsb)
                for j in range(NQ):
                    for h in range(4):
                        sl = slice(h * (QP // 4), (h + 1) * (QP // 4))
                        nc.sync.dma_start(out=xsb[32 * j:32 * j + 8, sl],
                                          in_=xin.ap()[sp, j, :, sl])
                # stationary block-diag palette, replicated per row strip
                palt_sb = singles.tile([128, 128], mm1_dt, name=f"palt_sb{sp}")
                palt_sbs.append(palt_sb)
                for j in range(NQ):
                    nc.sync.dma_start(out=palt_sb[32 * j:32 * j + 8, :],
                                      in_=palt.ap()[sp])
            palw_sb = singles.tile([128, 8], e_dt)
            nc.sync.dma_start(out=palw_sb, in_=palw_hi.ap())
            if n_palw == 2:
                palw_lo_sb = singles.tile([128, 8], e_dt)
                nc.sync.dma_start(out=palw_lo_sb, in_=palw_lo.ap())

            for g in range(NG):
                psum2 = ps2.tile([128, 512], f32)
                for rsub in range(RPG):
                    r = g * RPG + rsub
                    # two half-round pipelines: quarters (2h, 2h+1) ->
                    # psum1h -> exp -> e_sb -> 8 weighted-sum matmuls
                    for h in range(2):
                        psum1h = ps1.tile([128, 2 * RN], f32)
                        for jj in range(2):
                            j = 2 * h + jj
                            nc.tensor.matmul(
                                out=psum1h[:, RN * jj:RN * (jj + 1)],
                                lhsT=palt_sb[32 * j:32 * j + 8, :],
                                rhs=xsb[32 * j:32 * j + 8,
                                        RN * r:RN * (r + 1)],
                                start=True, stop=True,
                                tile_position=(32 * j, 0),
                            )
                        e_sb = epool.tile([128, 2 * RN], e_dt)
                        nc.scalar.activation(
                            out=e_sb, in_=psum1h,
                            func=mybir.ActivationFunctionType.Exp,
                            scale=float(scale),
                        )
                        for t in range(8):
                            s = 16 * rsub + 8 * h + t
                            lhsT = e_sb[:, 128 * t:128 * (t + 1)]
                            nc.tensor.matmul(
                                out=psum2[:, 8 * s:8 * s + 8],
                                lhsT=lhsT, rhs=palw_sb,
                                start=True, stop=(n_palw == 1),
                            )
                            if n_palw == 2:
                                nc.tensor.matmul(
                                    out=psum2[:, 8 * s:8 * s + 8],
                                    lhsT=lhsT, rhs=palw_lo_sb,
                                    start=False, stop=True,
                                )

                # divide numerators by denominators; write per-image blocks
                psr = psum2.rearrange("p (s e) -> p s e", e=8)
                recA = vpool.tile([128, 64], f32)
                nc.vector.reciprocal(out=recA, in_=psr[:, :, 3])
                recB = vpool.tile([128, 64], f32)
                nc.vector.reciprocal(out=recB, in_=psr[:, :, 7])
                outA = opool.tile([128, 192], f32)
                outB = opool.tile([128, 192], f32)
                oA = outA.rearrange("p (u c) -> p u c", c=3)
                oB = outB.rearrange("p (u c) -> p u c", c=3)
                for c in range(3):
                    nc.vector.tensor_mul(out=oA[:, :, c], in0=psr[:, :, c],
                                         in1=recA)
                    nc.vector.tensor_mul(out=oB[:, :, c], in0=psr[:, :, 4 + c],
                                         in1=recB)
                nc.sync.dma_start(out=out.ap()[0, g], in_=outA)
                nc.sync.dma_start(out=out.ap()[1, g], in_=outB)

    nc.compile()
    return nc


def _host_prep(images, palettes):
    """Per-core input arrays. images [16,256,256,3] f32, palettes [16,64,3]."""
    import ml_dtypes

    imgs = np.ascontiguousarray(images, np.float32).reshape(B, P, C)
    pals = np.ascontiguousarray(palettes, np.float32)
    in_maps = []
    for core in range(NCORES):
        ia, ib = imgs[2 * core], imgs[2 * core + 1]
        xin = np.empty((NQ, 8, QP), np.float32)
        xq_a = ia.reshape(NQ, QP, C).transpose(0, 2, 1)
        xq_b = ib.reshape(NQ, QP, C).transpose(0, 2, 1)
        xin[:, 0:3] = xq_a
        xin[:, 3] = 1.0
        xin[:, 4:7] = xq_b
        xin[:, 7] = 1.0

        pa, pb = pals[2 * core], pals[2 * core + 1]
        palt = np.zeros((8, 128), np.float32)
        palt[0:3, 0:64] = pa.T
        palt[3, 0:64] = -0.5 * (pa * pa).sum(-1)
        palt[4:7, 64:128] = pb.T
        palt[7, 64:128] = -0.5 * (pb * pb).sum(-1)

        palw = np.zeros((128, 8), np.float32)
        palw[0:64, 0:3] = pa
        palw[0:64, 3] = 1.0
        palw[64:128, 4:7] = pb
        palw[64:128, 7] = 1.0

        np_mm1 = {"float16": np.float16,
                  "bfloat16": ml_dtypes.bfloat16}.get(MM1_DT, np.float32)
        np_e = {"float16": np.float16,
                "bfloat16": ml_dtypes.bfloat16}.get(E_DT, np.float32)
        m = {"xin": xin.astype(np_mm1), "palt": palt.astype(np_mm1)}
        hi = palw.astype(np_e)
        m["palw_hi"] = hi
        if PALW_SPLIT and E_DT != "float32":
            m["palw_lo"] = (palw - hi.astype(np.float32)).astype(np_e)
        in_maps.append(m)
    return in_maps


def _host_post(results):
    """results[core]["out"] [2, 8, 128, 192] -> [16, 256, 256, 3]."""
    out = np.empty((B, P, C), np.float32)
    for core in range(NCORES):
        o = results[core]["out"]
        # out block col axis (64) decodes as (rsub 4, h 2, tj 2, q 4);
        # quarter j = 2h + tj, round r = 4g + rsub,
        # pixel = j*16384 + r*512 + q*128 + row
        dec = (o.reshape(IMGS_PER_CORE, NG, 128, RPG, 2, 2, 4, C)
                .transpose(0, 4, 5, 1, 3, 6, 2, 7)
                .reshape(IMGS_PER_CORE, P, C))
        out[2 * core] = dec[0]
        out[2 * core + 1] = dec[1]
    return out.reshape(B, H, W, C)


_CACHE = {}


def _get_nc(scale: float):
    key = (round(float(scale), 12), MM1_DT, E_DT, PALW_SPLIT)
    if key not in _CACHE:
        _CACHE[key] = build_bass(scale)
    return _CACHE[key]


def kernel(images, palettes, temperature, _trace=False):
    scale = 2.0 / float(np.asarray(temperature))
    nc = _get_nc(scale)
    in_maps = _host_prep(images, palettes)
    res = run_bass_kernel_spmd(nc, in_maps, core_ids=list(range(NCORES)),
                               trace=_trace)
    out = _host_post(res.results)
    if _trace:
        kernel.last_result = res
    return out
